# revision 21
# baseline (speedup 1.0000x reference)
"""Self-contained Trainium2 Bass kernel for nn_MoEWithDeepEP (8 NeuronCores).

Single launch per call:
  - Router (0.5 GFLOP of the model's ~40 GFLOP) runs on host in exact fp32,
    giving bit-identical top-2 selection to the reference; host also does the
    all-to-all dispatch bookkeeping (DeepEP's role), building per-slot
    pre-gathered token buffers.
  - The device kernel computes the shared expert (data-parallel over N/8
    tokens per core) plus 8 load-balanced expert slots per core.  Slot
    token-widths are a static template; the host assigns experts (sorted by
    load) to (core, slot) pairs so every expert fits, placing that expert's
    weights in the slot's weight buffer.  This removes ~45% padding waste vs
    a fixed per-expert capacity.
  - Gating + scatter-add combine run on host.
All DRAM tensors are pre-tiled [128, ...] partition-major so DMA descriptors
are 2-4KB.
"""
import sys
for _p in ("/opt/trn_rl_repo", "/root/.axon_site/_ro/trn_rl_repo"):
    if _p not in sys.path:
        sys.path.insert(0, _p)

import numpy as np

N = 8192          # tokens
D = 512           # model dim
E = 64            # experts
K = 2             # top-k
H = 256           # expert hidden
HS = 512          # shared hidden (H * NSH)
NCORES = 8
NS = N // NCORES      # tokens per core (data-parallel dim)
ROUTE_SCALE = 2.5
# static slot token-widths (descending); sized from the fixed-seed expert
# load distribution (rank maxes 390/297/283/268/244/238/225/207) + margin
TPS = (228, 392, 300, 288, 272, 248, 240, 208)
NSLOT = len(TPS)
NTILES = tuple(-(-w // 128) for w in TPS)


def _mk_bacc():
    from concourse import bacc

    return bacc.Bacc(
        "TRN2",
        target_bir_lowering=False,
        debug=False,
        enable_asserts=False,
        num_devices=NCORES,
    )


def build_kernel():
    """Shared expert + 8 expert slots (SwiGLU GEMMs, fp16)."""
    import concourse.bass as bass
    import concourse.tile as tile
    from concourse import mybir

    dt = mybir.dt
    AF = mybir.ActivationFunctionType
    OP = mybir.AluOpType
    nc = _mk_bacc()

    xsg = nc.dram_tensor("xsg", [128, 2, 4, 512], dt.float16, kind="ExternalInput")
    sw1 = nc.dram_tensor("sw1", [128, 4, HS], dt.float16, kind="ExternalInput")
    sw3 = nc.dram_tensor("sw3", [128, 4, HS], dt.float16, kind="ExternalInput")
    sw2 = nc.dram_tensor("sw2", [128, 4, D], dt.float16, kind="ExternalInput")
    xe_t = [
        nc.dram_tensor(f"xe{s}", [128, 4, TPS[s]], dt.float16, kind="ExternalInput")
        for s in range(NSLOT)
    ]
    w1_t = [
        nc.dram_tensor(f"w1_{s}", [128, 4, H], dt.float16, kind="ExternalInput")
        for s in range(NSLOT)
    ]
    w3_t = [
        nc.dram_tensor(f"w3_{s}", [128, 4, H], dt.float16, kind="ExternalInput")
        for s in range(NSLOT)
    ]
    w2_t = [
        nc.dram_tensor(f"w2_{s}", [128, 2, D], dt.float16, kind="ExternalInput")
        for s in range(NSLOT)
    ]

    ysh_out = nc.dram_tensor("ysh_out", [128, NS // 128, D], dt.float16,
                             kind="ExternalOutput")
    y_t = [
        nc.dram_tensor(f"y{s}", [128, NTILES[s], D], dt.float16,
                       kind="ExternalOutput")
        for s in range(NSLOT)
    ]

    with tile.TileContext(nc) as tc:
        with (
            tc.tile_pool(name="const", bufs=1) as cpool,
            tc.tile_pool(name="hps", bufs=4, space="PSUM") as hpsum,
            tc.tile_pool(name="yps", bufs=4, space="PSUM") as ypsum,
            tc.tile_pool(name="ew", bufs=3) as ewpool,
            tc.tile_pool(name="work", bufs=3) as wpool,
        ):
            xs_sb = cpool.tile([128, 2, 4, 512], dt.float16)
            sw1_sb = cpool.tile([128, 4, HS], dt.float16)
            sw3_sb = cpool.tile([128, 4, HS], dt.float16)
            sw2_sb = cpool.tile([128, 4, D], dt.float16)
            ysh_sb = cpool.tile([128, NS // 128, D], dt.float16)

            slot_sb = {}

            def load_slot(s):
                b = "b" if s >= 4 else ""
                w1_sb = ewpool.tile([128, 4, H], dt.float16, tag="w1" + b)
                nc.sync.dma_start(w1_sb[:], w1_t[s].ap())
                xe = wpool.tile([128, 4, TPS[s]], dt.float16, tag="xe" + b)
                nc.sync.dma_start(xe[:], xe_t[s].ap())
                w3_sb = ewpool.tile([128, 4, H], dt.float16, tag="w3" + b)
                nc.sync.dma_start(w3_sb[:], w3_t[s].ap())
                w2_sb = ewpool.tile([128, 2, D], dt.float16, tag="w2" + b)
                nc.sync.dma_start(w2_sb[:], w2_t[s].ap())
                slot_sb[s] = (w1_sb, w3_sb, w2_sb, xe)

            def do_slot(s):
                W = TPS[s]
                nt = NTILES[s]
                w1_sb, w3_sb, w2_sb, xe = slot_sb.pop(s)
                he = wpool.tile([128, 2, W], dt.float16, tag="he")
                for hc in range(2):
                    ph1 = hpsum.tile([128, W], dt.float32, tag="ph")
                    for c in range(4):
                        nc.tensor.matmul(
                            ph1[:], lhsT=w1_sb[:, c, bass.ts(hc, 128)],
                            rhs=xe[:, c, :], start=(c == 0), stop=(c == 3),
                        )
                    ph3 = hpsum.tile([128, W], dt.float32, tag="ph")
                    for c in range(4):
                        nc.tensor.matmul(
                            ph3[:], lhsT=w3_sb[:, c, bass.ts(hc, 128)],
                            rhs=xe[:, c, :], start=(c == 0), stop=(c == 3),
                        )
                    t1 = wpool.tile([128, W], dt.float32, tag="silu")
                    nc.scalar.activation(t1[:], ph1[:], AF.Silu)
                    nc.vector.tensor_tensor(
                        out=he[:, hc, :], in0=t1[:], in1=ph3[:], op=OP.mult
                    )
                yb = wpool.tile([128, nt, D], dt.float16, tag="yb")
                for tc_ in range(nt):
                    w = min(128, W - tc_ * 128)
                    py = ypsum.tile([128, D], dt.float32, tag="py")
                    for hc in range(2):
                        nc.tensor.matmul(
                            py[0:w, :],
                            lhsT=he[:, hc, tc_ * 128:tc_ * 128 + w],
                            rhs=w2_sb[:, hc, :], start=(hc == 0), stop=(hc == 1),
                        )
                    if tc_ % 2 == 0:
                        nc.scalar.copy(yb[0:w, tc_, :], py[0:w, :])
                    else:
                        nc.vector.tensor_copy(yb[0:w, tc_, :], py[0:w, :])
                nc.scalar.dma_start(y_t[s].ap(), yb[:])

            def do_shared(g):
                hsh = wpool.tile([128, 4, 512], dt.float16, tag="hsh")
                for hc in range(4):
                    ph1 = hpsum.tile([128, 512], dt.float32, tag="ph")
                    for c in range(4):
                        nc.tensor.matmul(
                            ph1[:], lhsT=sw1_sb[:, c, bass.ts(hc, 128)],
                            rhs=xs_sb[:, g, c, :], start=(c == 0), stop=(c == 3),
                        )
                    ph3 = hpsum.tile([128, 512], dt.float32, tag="ph")
                    for c in range(4):
                        nc.tensor.matmul(
                            ph3[:], lhsT=sw3_sb[:, c, bass.ts(hc, 128)],
                            rhs=xs_sb[:, g, c, :], start=(c == 0), stop=(c == 3),
                        )
                    t1 = wpool.tile([128, 512], dt.float32, tag="silu")
                    nc.scalar.activation(t1[:], ph1[:], AF.Silu)
                    nc.vector.tensor_tensor(
                        out=hsh[:, hc, :], in0=t1[:], in1=ph3[:], op=OP.mult
                    )
                for tc_ in range(4):
                    py = ypsum.tile([128, D], dt.float32, tag="py")
                    for hc in range(4):
                        nc.tensor.matmul(
                            py[:], lhsT=hsh[:, hc, bass.ts(tc_, 128)],
                            rhs=sw2_sb[:, hc, :], start=(hc == 0), stop=(hc == 3),
                        )
                    if tc_ % 2 == 0:
                        nc.scalar.copy(ysh_sb[:, g * 4 + tc_, :], py[:])
                    else:
                        nc.vector.tensor_copy(ysh_sb[:, g * 4 + tc_, :], py[:])
                nc.scalar.dma_start(
                    ysh_out.ap()[:, g * 4:(g + 1) * 4, :],
                    ysh_sb[:, g * 4:(g + 1) * 4, :],
                )

            # PE p-state warm-up: the tensor clock ramps 1.2->2.4GHz over
            # ~3us of continuous work; run dummy matmuls on a zeroed tile
            # while the first input DMAs are still in flight.
            warm = cpool.tile([128, 512], dt.float16)
            nc.vector.memset(warm[:], 0.0)
            wps = hpsum.tile([128, 512], dt.float32, tag="ph")
            for _ in range(8):
                nc.tensor.matmul(wps[0:64, :], lhsT=warm[:, 0:64], rhs=warm[:])

            # DMA issue order = need order: slot0 | shared-g0 deps | slot1 |
            # shared-g1 deps | slots 2..7.  Stores go on the Act HWDGE queue
            # (scalar.dma_start) so they never block loads.
            load_slot(0)
            load_slot(1)
            nc.sync.dma_start(xs_sb[:, 0], xsg.ap()[:, 0])
            nc.sync.dma_start(sw1_sb[:], sw1.ap())
            nc.sync.dma_start(sw3_sb[:], sw3.ap())
            load_slot(2)
            nc.sync.dma_start(xs_sb[:, 1], xsg.ap()[:, 1])
            nc.sync.dma_start(sw2_sb[:], sw2.ap())
            load_slot(3)
            do_slot(0)
            do_slot(1)
            load_slot(4)
            do_shared(0)
            load_slot(5)
            do_slot(2)
            load_slot(6)
            do_slot(3)
            load_slot(7)
            do_shared(1)
            for s in range(4, NSLOT):
                do_slot(s)

    nc.compile()
    return nc


# ---------------- host: router, dispatch, combine ----------------

def _tile_pd(a, np_dt):
    """[P*128, F] -> [128, P, F] partition-major pre-tiled."""
    p = a.shape[0] // 128
    return np.ascontiguousarray(
        a.reshape(p, 128, a.shape[1]).transpose(1, 0, 2).astype(np_dt)
    )


def host_route(x, gate_w):
    """Exact fp32 router + top-2 + normalized gating (reference math)."""
    xf = np.asarray(x, np.float32).reshape(N, D)
    logits = xf @ np.asarray(gate_w, np.float32).T          # [N, E]
    part = np.argpartition(-logits, K - 1, axis=1)[:, :K]
    vals = np.take_along_axis(logits, part, axis=1)
    order = np.argsort(-vals, axis=1, kind="stable")
    top_idx = np.take_along_axis(part, order, axis=1)       # [N, K]
    top_vals = np.take_along_axis(vals, order, axis=1)
    scores = 1.0 / (1.0 + np.exp(-top_vals))
    gates = scores / (scores.sum(1, keepdims=True) + 1e-20) * ROUTE_SCALE

    flat_e = top_idx.reshape(-1)
    order_p = np.argsort(flat_e, kind="stable")
    counts = np.bincount(flat_e, minlength=E)
    splits = np.split(order_p, np.cumsum(counts)[:-1])
    gflat = gates.reshape(-1).astype(np.float32)
    toks_l = [(pr // K).astype(np.int64) for pr in splits]
    gates_l = [gflat[pr] for pr in splits]
    return toks_l, gates_l


def assign_slots(toks_l, gates_l):
    """Greedy: biggest remaining expert chunk -> biggest remaining slot.

    Returns assign[c][s] = (expert_id, tokens, gates); experts larger than a
    slot are split across slots (weights duplicated by the host).
    """
    import heapq

    slots = sorted(
        ((TPS[s], c, s) for c in range(NCORES) for s in range(NSLOT)),
        key=lambda t: -t[0],
    )
    heap = [(-len(t), ge, 0) for ge, t in enumerate(toks_l) if len(t)]
    heapq.heapify(heap)
    assign = [[None] * NSLOT for _ in range(NCORES)]
    for size, c, s in slots:
        if not heap:
            assign[c][s] = (0, np.empty(0, np.int64), np.empty(0, np.float32))
            continue
        negn, ge, off = heapq.heappop(heap)
        n = -negn
        take = min(n, size)
        assign[c][s] = (ge, toks_l[ge][off:off + take], gates_l[ge][off:off + take])
        if n > take:
            heapq.heappush(heap, (-(n - take), ge, off + take))
    if heap:
        raise RuntimeError("slot capacity exceeded; enlarge TPS")
    return assign


def host_prepare(x, w1, w3, w2, sw1, sw3, sw2, assign):
    xf16 = np.asarray(x, np.float32).reshape(N, D).astype(np.float16)
    w1h = np.asarray(w1, np.float32).astype(np.float16)
    w3h = np.asarray(w3, np.float32).astype(np.float16)
    w2h = np.asarray(w2, np.float32).astype(np.float16)
    sw1t = _tile_pd(np.asarray(sw1, np.float32), np.float16)
    sw3t = _tile_pd(np.asarray(sw3, np.float32), np.float16)
    sw2t = _tile_pd(np.asarray(sw2, np.float32), np.float16)
    in_maps = []
    for c in range(NCORES):
        xT = xf16[c * NS:(c + 1) * NS].T                  # [D, NS]
        xsg = np.ascontiguousarray(
            xT.reshape(4, 128, 2, 512).transpose(1, 2, 0, 3)
        )                                                  # [128, 2, 4, 512]
        im = {"xsg": xsg, "sw1": sw1t, "sw3": sw3t, "sw2": sw2t}
        for s in range(NSLOT):
            ge, toks, _ = assign[c][s]
            ids = np.zeros(TPS[s], np.int64)
            ids[:len(toks)] = toks
            xeT = xf16[ids].T                              # [D, W]
            im[f"xe{s}"] = np.ascontiguousarray(
                xeT.reshape(4, 128, TPS[s]).transpose(1, 0, 2)
            )
            im[f"w1_{s}"] = np.ascontiguousarray(
                w1h[ge].reshape(4, 128, H).transpose(1, 0, 2)
            )
            im[f"w3_{s}"] = np.ascontiguousarray(
                w3h[ge].reshape(4, 128, H).transpose(1, 0, 2)
            )
            im[f"w2_{s}"] = np.ascontiguousarray(
                w2h[ge].reshape(2, 128, D).transpose(1, 0, 2)
            )
        in_maps.append(im)
    return in_maps


def host_combine(res, assign):
    out = np.zeros((N, D), dtype=np.float32)
    for c, r in enumerate(res):
        ysh = r["ysh_out"].transpose(1, 0, 2).reshape(NS, D)
        out[c * NS:(c + 1) * NS] += ysh.astype(np.float32)
        for s in range(NSLOT):
            _, toks, gates = assign[c][s]
            n = len(toks)
            if not n:
                continue
            y = r[f"y{s}"].transpose(1, 0, 2).reshape(-1, D)[:n]
            out[toks] += y.astype(np.float32) * gates[:, None]
    return out.reshape(4, 2048, D)


_CACHE = {}


def kernel(x, gate_w, w1, w3, w2, sw1, sw3, sw2):
    from concourse.bass_utils import run_bass_kernel_spmd

    if "nc" not in _CACHE:
        _CACHE["nc"] = build_kernel()
    nc = _CACHE["nc"]

    toks_l, gates_l = host_route(x, gate_w)
    assign = assign_slots(toks_l, gates_l)
    in_maps = host_prepare(x, w1, w3, w2, sw1, sw3, sw2, assign)
    res = run_bass_kernel_spmd(
        nc, in_maps, core_ids=list(range(NCORES))
    ).results
    return host_combine(res, assign).astype(np.float32)


# revision 22
# speedup vs baseline: 1.0387x; 1.0387x over previous
"""Self-contained Trainium2 Bass kernel for nn_MoEWithDeepEP (8 NeuronCores).

Single launch per call:
  - Router (0.5 GFLOP of the model's ~40 GFLOP) runs on host in exact fp32,
    giving bit-identical top-2 selection to the reference; host also does the
    all-to-all dispatch bookkeeping (DeepEP's role), building per-slot
    pre-gathered token buffers.
  - The device kernel computes the shared expert (data-parallel over N/8
    tokens per core) plus 8 load-balanced expert slots per core.  Slot
    token-widths are a static template; the host assigns experts (sorted by
    load) to (core, slot) pairs so every expert fits, placing that expert's
    weights in the slot's weight buffer.  This removes ~45% padding waste vs
    a fixed per-expert capacity.
  - Gating + scatter-add combine run on host.
All DRAM tensors are pre-tiled [128, ...] partition-major so DMA descriptors
are 2-4KB.
"""
import sys
for _p in ("/opt/trn_rl_repo", "/root/.axon_site/_ro/trn_rl_repo"):
    if _p not in sys.path:
        sys.path.insert(0, _p)

import numpy as np

N = 8192          # tokens
D = 512           # model dim
E = 64            # experts
K = 2             # top-k
H = 256           # expert hidden
HS = 512          # shared hidden (H * NSH)
NCORES = 8
NS = N // NCORES      # tokens per core (data-parallel dim)
ROUTE_SCALE = 2.5
# static slot token-widths (descending); sized from the fixed-seed expert
# load distribution (rank maxes 390/297/283/268/244/238/225/207) + margin
TPS = (228, 392, 300, 288, 272, 248, 240, 208)
NSLOT = len(TPS)
NTILES = tuple(-(-w // 128) for w in TPS)


def _mk_bacc():
    from concourse import bacc

    return bacc.Bacc(
        "TRN2",
        target_bir_lowering=False,
        debug=False,
        enable_asserts=False,
        num_devices=NCORES,
    )


def build_kernel():
    """Shared expert + 8 expert slots (SwiGLU GEMMs, fp16)."""
    import concourse.bass as bass
    import concourse.tile as tile
    from concourse import mybir

    dt = mybir.dt
    AF = mybir.ActivationFunctionType
    OP = mybir.AluOpType
    nc = _mk_bacc()

    xsg = nc.dram_tensor("xsg", [128, 2, 4, 512], dt.float16, kind="ExternalInput")
    sw1 = nc.dram_tensor("sw1", [128, 4, HS], dt.float16, kind="ExternalInput")
    sw3 = nc.dram_tensor("sw3", [128, 4, HS], dt.float16, kind="ExternalInput")
    sw2 = nc.dram_tensor("sw2", [128, 4, D], dt.float16, kind="ExternalInput")
    xe_t = [
        nc.dram_tensor(f"xe{s}", [128, 4, TPS[s]], dt.float16, kind="ExternalInput")
        for s in range(NSLOT)
    ]
    w1_t = [
        nc.dram_tensor(f"w1_{s}", [128, 4, H], dt.float16, kind="ExternalInput")
        for s in range(NSLOT)
    ]
    w3_t = [
        nc.dram_tensor(f"w3_{s}", [128, 4, H], dt.float16, kind="ExternalInput")
        for s in range(NSLOT)
    ]
    w2_t = [
        nc.dram_tensor(f"w2_{s}", [128, 2, D], dt.float16, kind="ExternalInput")
        for s in range(NSLOT)
    ]

    ysh_out = nc.dram_tensor("ysh_out", [128, NS // 128, D], dt.float16,
                             kind="ExternalOutput")
    y_t = [
        nc.dram_tensor(f"y{s}", [128, NTILES[s], D], dt.float16,
                       kind="ExternalOutput")
        for s in range(NSLOT)
    ]

    with tile.TileContext(nc) as tc:
        with (
            tc.tile_pool(name="const", bufs=1) as cpool,
            tc.tile_pool(name="hps", bufs=4, space="PSUM") as hpsum,
            tc.tile_pool(name="yps", bufs=4, space="PSUM") as ypsum,
            tc.tile_pool(name="ew", bufs=3) as ewpool,
            tc.tile_pool(name="work", bufs=3) as wpool,
        ):
            xs_sb = cpool.tile([128, 2, 4, 512], dt.float16)
            sw1_sb = cpool.tile([128, 4, HS], dt.float16)
            sw3_sb = cpool.tile([128, 4, HS], dt.float16)
            sw2_sb = cpool.tile([128, 4, D], dt.float16)
            ysh_sb = cpool.tile([128, NS // 128, D], dt.float16)

            slot_sb = {}

            def load_slot(s):
                b = "b" if s >= 4 else ""
                w1_sb = ewpool.tile([128, 4, H], dt.float16, tag="w1" + b)
                nc.sync.dma_start(w1_sb[:], w1_t[s].ap())
                xe = wpool.tile([128, 4, TPS[s]], dt.float16, tag="xe" + b)
                nc.sync.dma_start(xe[:], xe_t[s].ap())
                w3_sb = ewpool.tile([128, 4, H], dt.float16, tag="w3" + b)
                nc.sync.dma_start(w3_sb[:], w3_t[s].ap())
                w2_sb = ewpool.tile([128, 2, D], dt.float16, tag="w2" + b)
                nc.sync.dma_start(w2_sb[:], w2_t[s].ap())
                slot_sb[s] = (w1_sb, w3_sb, w2_sb, xe)

            def do_slot(s):
                W = TPS[s]
                nt = NTILES[s]
                w1_sb, w3_sb, w2_sb, xe = slot_sb.pop(s)
                he = wpool.tile([128, 2, W], dt.float16, tag="he")
                for hc in range(2):
                    ph1 = hpsum.tile([128, W], dt.float32, tag="ph")
                    for c in range(4):
                        nc.tensor.matmul(
                            ph1[:], lhsT=w1_sb[:, c, bass.ts(hc, 128)],
                            rhs=xe[:, c, :], start=(c == 0), stop=(c == 3),
                        )
                    ph3 = hpsum.tile([128, W], dt.float32, tag="ph")
                    for c in range(4):
                        nc.tensor.matmul(
                            ph3[:], lhsT=w3_sb[:, c, bass.ts(hc, 128)],
                            rhs=xe[:, c, :], start=(c == 0), stop=(c == 3),
                        )
                    t1 = wpool.tile([128, W], dt.float32, tag="silu")
                    nc.scalar.activation(t1[:], ph1[:], AF.Silu)
                    nc.vector.tensor_tensor(
                        out=he[:, hc, :], in0=t1[:], in1=ph3[:], op=OP.mult
                    )
                yb = wpool.tile([128, nt, D], dt.float16, tag="yb")
                for tc_ in range(nt):
                    w = min(128, W - tc_ * 128)
                    py = ypsum.tile([128, D], dt.float32, tag="py")
                    for hc in range(2):
                        nc.tensor.matmul(
                            py[0:w, :],
                            lhsT=he[:, hc, tc_ * 128:tc_ * 128 + w],
                            rhs=w2_sb[:, hc, :], start=(hc == 0), stop=(hc == 1),
                        )
                    if tc_ % 2 == 0:
                        nc.scalar.copy(yb[0:w, tc_, :], py[0:w, :])
                    else:
                        nc.vector.tensor_copy(yb[0:w, tc_, :], py[0:w, :])
                nc.scalar.dma_start(y_t[s].ap(), yb[:])

            def do_shared(g):
                hsh = wpool.tile([128, 4, 512], dt.float16, tag="hsh")
                for hc in range(4):
                    ph1 = hpsum.tile([128, 512], dt.float32, tag="ph")
                    for c in range(4):
                        nc.tensor.matmul(
                            ph1[:], lhsT=sw1_sb[:, c, bass.ts(hc, 128)],
                            rhs=xs_sb[:, g, c, :], start=(c == 0), stop=(c == 3),
                        )
                    ph3 = hpsum.tile([128, 512], dt.float32, tag="ph")
                    for c in range(4):
                        nc.tensor.matmul(
                            ph3[:], lhsT=sw3_sb[:, c, bass.ts(hc, 128)],
                            rhs=xs_sb[:, g, c, :], start=(c == 0), stop=(c == 3),
                        )
                    t1 = wpool.tile([128, 512], dt.float32, tag="silu")
                    nc.scalar.activation(t1[:], ph1[:], AF.Silu)
                    nc.vector.tensor_tensor(
                        out=hsh[:, hc, :], in0=t1[:], in1=ph3[:], op=OP.mult
                    )
                for tc_ in range(4):
                    py = ypsum.tile([128, D], dt.float32, tag="py")
                    for hc in range(4):
                        nc.tensor.matmul(
                            py[:], lhsT=hsh[:, hc, bass.ts(tc_, 128)],
                            rhs=sw2_sb[:, hc, :], start=(hc == 0), stop=(hc == 3),
                        )
                    if tc_ % 2 == 0:
                        nc.scalar.copy(ysh_sb[:, g * 4 + tc_, :], py[:])
                    else:
                        nc.vector.tensor_copy(ysh_sb[:, g * 4 + tc_, :], py[:])
                nc.scalar.dma_start(
                    ysh_out.ap()[:, g * 4:(g + 1) * 4, :],
                    ysh_sb[:, g * 4:(g + 1) * 4, :],
                )

            # PE p-state warm-up: the tensor clock ramps 1.2->2.4GHz over
            # ~3us of continuous work; run dummy matmuls on a zeroed tile
            # while the first input DMAs are still in flight.
            warm = cpool.tile([128, 512], dt.float16)
            nc.vector.memset(warm[:], 0.0)
            wps = hpsum.tile([128, 512], dt.float32, tag="ph")
            for _ in range(10):
                nc.tensor.matmul(wps[0:64, :], lhsT=warm[:, 0:64], rhs=warm[:])

            # DMA issue order = need order: slot0 | shared-g0 deps | slot1 |
            # shared-g1 deps | slots 2..7.  Stores go on the Act HWDGE queue
            # (scalar.dma_start) so they never block loads.
            load_slot(0)
            load_slot(1)
            nc.sync.dma_start(xs_sb[:, 0], xsg.ap()[:, 0])
            nc.sync.dma_start(sw1_sb[:], sw1.ap())
            nc.sync.dma_start(sw3_sb[:], sw3.ap())
            load_slot(2)
            nc.sync.dma_start(xs_sb[:, 1], xsg.ap()[:, 1])
            nc.sync.dma_start(sw2_sb[:], sw2.ap())
            load_slot(3)
            do_slot(0)
            do_slot(1)
            load_slot(4)
            do_shared(0)
            load_slot(5)
            do_slot(2)
            load_slot(6)
            do_slot(3)
            load_slot(7)
            do_shared(1)
            for s in range(4, NSLOT):
                do_slot(s)

    nc.compile()
    return nc


# ---------------- host: router, dispatch, combine ----------------

def _tile_pd(a, np_dt):
    """[P*128, F] -> [128, P, F] partition-major pre-tiled."""
    p = a.shape[0] // 128
    return np.ascontiguousarray(
        a.reshape(p, 128, a.shape[1]).transpose(1, 0, 2).astype(np_dt)
    )


def host_route(x, gate_w):
    """Exact fp32 router + top-2 + normalized gating (reference math)."""
    xf = np.asarray(x, np.float32).reshape(N, D)
    logits = xf @ np.asarray(gate_w, np.float32).T          # [N, E]
    part = np.argpartition(-logits, K - 1, axis=1)[:, :K]
    vals = np.take_along_axis(logits, part, axis=1)
    order = np.argsort(-vals, axis=1, kind="stable")
    top_idx = np.take_along_axis(part, order, axis=1)       # [N, K]
    top_vals = np.take_along_axis(vals, order, axis=1)
    scores = 1.0 / (1.0 + np.exp(-top_vals))
    gates = scores / (scores.sum(1, keepdims=True) + 1e-20) * ROUTE_SCALE

    flat_e = top_idx.reshape(-1)
    order_p = np.argsort(flat_e, kind="stable")
    counts = np.bincount(flat_e, minlength=E)
    splits = np.split(order_p, np.cumsum(counts)[:-1])
    gflat = gates.reshape(-1).astype(np.float32)
    toks_l = [(pr // K).astype(np.int64) for pr in splits]
    gates_l = [gflat[pr] for pr in splits]
    return toks_l, gates_l


def assign_slots(toks_l, gates_l):
    """Greedy: biggest remaining expert chunk -> biggest remaining slot.

    Returns assign[c][s] = (expert_id, tokens, gates); experts larger than a
    slot are split across slots (weights duplicated by the host).
    """
    import heapq

    slots = sorted(
        ((TPS[s], c, s) for c in range(NCORES) for s in range(NSLOT)),
        key=lambda t: -t[0],
    )
    heap = [(-len(t), ge, 0) for ge, t in enumerate(toks_l) if len(t)]
    heapq.heapify(heap)
    assign = [[None] * NSLOT for _ in range(NCORES)]
    for size, c, s in slots:
        if not heap:
            assign[c][s] = (0, np.empty(0, np.int64), np.empty(0, np.float32))
            continue
        negn, ge, off = heapq.heappop(heap)
        n = -negn
        take = min(n, size)
        assign[c][s] = (ge, toks_l[ge][off:off + take], gates_l[ge][off:off + take])
        if n > take:
            heapq.heappush(heap, (-(n - take), ge, off + take))
    if heap:
        raise RuntimeError("slot capacity exceeded; enlarge TPS")
    return assign


def host_prepare(x, w1, w3, w2, sw1, sw3, sw2, assign):
    xf16 = np.asarray(x, np.float32).reshape(N, D).astype(np.float16)
    w1h = np.asarray(w1, np.float32).astype(np.float16)
    w3h = np.asarray(w3, np.float32).astype(np.float16)
    w2h = np.asarray(w2, np.float32).astype(np.float16)
    sw1t = _tile_pd(np.asarray(sw1, np.float32), np.float16)
    sw3t = _tile_pd(np.asarray(sw3, np.float32), np.float16)
    sw2t = _tile_pd(np.asarray(sw2, np.float32), np.float16)
    in_maps = []
    for c in range(NCORES):
        xT = xf16[c * NS:(c + 1) * NS].T                  # [D, NS]
        xsg = np.ascontiguousarray(
            xT.reshape(4, 128, 2, 512).transpose(1, 2, 0, 3)
        )                                                  # [128, 2, 4, 512]
        im = {"xsg": xsg, "sw1": sw1t, "sw3": sw3t, "sw2": sw2t}
        for s in range(NSLOT):
            ge, toks, _ = assign[c][s]
            ids = np.zeros(TPS[s], np.int64)
            ids[:len(toks)] = toks
            xeT = xf16[ids].T                              # [D, W]
            im[f"xe{s}"] = np.ascontiguousarray(
                xeT.reshape(4, 128, TPS[s]).transpose(1, 0, 2)
            )
            im[f"w1_{s}"] = np.ascontiguousarray(
                w1h[ge].reshape(4, 128, H).transpose(1, 0, 2)
            )
            im[f"w3_{s}"] = np.ascontiguousarray(
                w3h[ge].reshape(4, 128, H).transpose(1, 0, 2)
            )
            im[f"w2_{s}"] = np.ascontiguousarray(
                w2h[ge].reshape(2, 128, D).transpose(1, 0, 2)
            )
        in_maps.append(im)
    return in_maps


def host_combine(res, assign):
    out = np.zeros((N, D), dtype=np.float32)
    for c, r in enumerate(res):
        ysh = r["ysh_out"].transpose(1, 0, 2).reshape(NS, D)
        out[c * NS:(c + 1) * NS] += ysh.astype(np.float32)
        for s in range(NSLOT):
            _, toks, gates = assign[c][s]
            n = len(toks)
            if not n:
                continue
            y = r[f"y{s}"].transpose(1, 0, 2).reshape(-1, D)[:n]
            out[toks] += y.astype(np.float32) * gates[:, None]
    return out.reshape(4, 2048, D)


_CACHE = {}


def kernel(x, gate_w, w1, w3, w2, sw1, sw3, sw2):
    from concourse.bass_utils import run_bass_kernel_spmd

    if "nc" not in _CACHE:
        _CACHE["nc"] = build_kernel()
    nc = _CACHE["nc"]

    toks_l, gates_l = host_route(x, gate_w)
    assign = assign_slots(toks_l, gates_l)
    in_maps = host_prepare(x, w1, w3, w2, sw1, sw3, sw2, assign)
    res = run_bass_kernel_spmd(
        nc, in_maps, core_ids=list(range(NCORES))
    ).results
    return host_combine(res, assign).astype(np.float32)
